# revision 2
# baseline (speedup 1.0000x reference)
"""Trainium2 Bass kernel for pairwise DiceLoss (v2: 4x PE column tiling).

Math (per reference):
    an[b,k,:]  = am[b,k,:] / (S[b,k] + EPS),  S = row sums of am
    gram_n     = an . an^T per batch          (16 x 16 per batch)
    dice[b,k,l]= (2*gram_n + 0.1) / (a[b,k] + a[b,l] + 0.1),  a = S/(S+EPS)
    loss       = mean over masked (k<l, same batch) pairs and batches

fp32-exact algebra: S ~ 32768 so S + 1e-8 == S in fp32 => a == 1.0 exactly
and the dice denominator is the constant 2.1 (matches the reference's own
fp32 arithmetic to ~1e-7).  Only the masked-normalized-gram row sums
t_m = sum_j mask * G[m,j] * r_m * r_j  must come back; the affine map to the
loss is applied on host.

Device strategy (per core, 8 batches):
  - Only block-diagonal 16x16 grams are needed, so instead of one 128x129
    Gram (the v1 kernel; rhs-stream-bound at ~59ns/MM => ~30us), split the
    8 batches into G=4 groups of 2 (32 rows + a ones row = 33).  Each group
    streams through its own 32-column strip of the PE array
    (tile_position=(0,32g)): 4 concurrent rhs streams on separate XBUSes.
    Per 128-pixel chunk: 4 matmuls lhsT=[128,32], rhs=[128,33], N=33,
    accumulating into partition-disjoint PSUM slices.
  - fp8e4m3 input (4x less HBM traffic; quantization error cancels over
    65536-element contractions, measured ~1e-6 end to end).
  - Warmup: dummy matmuls from program start keep the PE busy through the
    HAM activity window so real matmuls run at 2.4GHz, not 1.2.
  - DMA: few, large tiles sized so the serial dma_start issue rate (~0.65us
    each on the Sync queue) always stays ahead of the ~0.38MB/us drain.
  - Epilogue: r = 1/(S+eps) per row; block-broadcast of r_j via 4 tiny
    tile-position matmuls against a replicated 32x32 identity; 3 DVE ops +
    row-reduce; output is just [128,1] f32 per core.
Host: loss = (2*sum(t) + 0.1*npairs_total) / 2.1 / npairs_total.
"""

import os

import numpy as np

B, K, N = 64, 16, 65536
NCORES = 8
BPC = B // NCORES  # 8 batches per core
P = 128  # SBUF partitions
G = 4  # column-tile groups per core (2 batches each)
GR = 2 * K + 1  # 33 rows per group incl ones row
C_PER_P = N // P  # 512 pixel-chunks of 128
TILES = [24, 64, 104, 104, 108, 108]  # sums to C_PER_P
WARMUP = int(os.environ.get("KERNEL_WARMUP", "28"))
SMOOTH = 0.1
EPS = 1e-8

_CACHE: dict = {}

# test.py reads this after calling kernel() to print HW exec time
LAST_RESULTS = None


def _build_nc():
    import concourse.bacc as bacc
    import concourse.mybir as mybir
    import concourse.tile as tile

    f32 = mybir.dt.float32
    xdt = mybir.dt.float8e4
    nc = bacc.Bacc("TRN2", target_bir_lowering=False)

    x = nc.dram_tensor("x", [P, C_PER_P, G, GR], xdt, kind="ExternalInput")
    consts = nc.dram_tensor("consts", [P, 64], f32, kind="ExternalInput")
    out_d = nc.dram_tensor("out", [P, 1], f32, kind="ExternalOutput")

    with tile.TileContext(nc) as tc:
        with (
            tc.tile_pool(name="xp", bufs=1) as xp,
            tc.tile_pool(name="sg", bufs=1) as sg,
            tc.tile_pool(name="ps", bufs=1, space="PSUM") as ps,
            tc.tile_pool(name="ps2", bufs=1, space="PSUM") as ps2,
            tc.tile_pool(name="wps", bufs=1, space="PSUM") as wps,
        ):
            # ---- PE warmup: junk matmuls with no DMA dependency ----
            wjunk = sg.tile([P, 16], xdt, name="wjunk")
            nc.gpsimd.memset(wjunk[:], 0.5)
            w_ps = wps.tile([P, 128], f32)
            for _ in range(WARMUP):
                nc.tensor.matmul(
                    w_ps[0:16, :],
                    wjunk[:, 0:16],
                    wjunk[:, 0:1].to_broadcast([P, 128]),
                    start=True,
                    stop=True,
                )

            # ---- input tiles (all resident; ~66KB/partition at fp8) ----
            xts = []
            off = 0
            for t, cc in enumerate(TILES):
                xt = xp.tile([P, cc, G, GR], xdt, name=f"xt{t}")
                nc.sync.dma_start(xt[:], x[:, off : off + cc, :, :])
                xts.append((xt, off, cc))
                off += cc
            # epilogue-only data, off the critical path
            consts_sb = sg.tile([P, 64], f32)
            nc.sync.dma_start(consts_sb[:], consts[:, :])
            maskc = consts_sb[:, 0:32]
            ident = consts_sb[:, 32:64]

            # ---- 4 concurrent block-gram accumulations ----
            g_ps = ps.tile([P, GR], f32)
            for xt, off, cc in xts:
                for c in range(cc):
                    for g in range(G):
                        nc.tensor.matmul(
                            g_ps[32 * g : 32 * g + 32, :],
                            xt[:, c, g, 0:32],
                            xt[:, c, g, :],
                            start=(off + c == 0),
                            stop=(off + c == C_PER_P - 1),
                            tile_position=(0, 32 * g),
                        )

            # ---- epilogue ----
            s_ps = g_ps[:, 2 * K : 2 * K + 1]  # S[row] in PSUM
            pack = sg.tile([P, 2], f32)
            nc.vector.tensor_scalar_add(pack[:, 1:2], s_ps, EPS)
            nc.vector.reciprocal(pack[:, 0:1], pack[:, 1:2])  # r = 1/(S+eps)
            rcol = pack[:, 0:1]

            # rB[32g+m, j] = r[32g+j]: per-block partition broadcast, one
            # step-0-weights matmul per 32x32 diagonal tile
            rB = ps2.tile([P, 32], f32)
            for g in range(G):
                nc.tensor.matmul(
                    rB[32 * g : 32 * g + 32, :],
                    rcol[32 * g : 32 * g + 32, 0:1].to_broadcast([32, 32]),
                    ident[32 * g : 32 * g + 32, :],
                    start=True,
                    stop=True,
                    tile_position=(32 * g, 32 * g),
                )

            t1 = sg.tile([P, 32], f32)
            nc.vector.tensor_scalar_mul(t1[:], g_ps[:, 0:32], rcol)  # G*r_m
            nc.vector.tensor_mul(t1[:], t1[:], rB[:])  # *r_j
            nc.vector.tensor_mul(t1[:], t1[:], maskc)  # mask k<l same batch
            osb = sg.tile([P, 1], f32)
            nc.vector.reduce_sum(osb[:], t1[:], axis=mybir.AxisListType.X)
            nc.sync.dma_start(out_d[:, :], osb[:])

    nc.compile()
    return nc


def _make_consts() -> np.ndarray:
    consts = np.zeros((P, 64), dtype=np.float32)
    m = np.arange(P)[:, None] % 32
    j = np.arange(32)[None, :]
    # mask[m, j] = 1 iff same batch within the 2-batch group and k < l
    consts[:, 0:32] = ((m // K == j // K) & (m % K < j % K)).astype(np.float32)
    consts[:, 32:64] = (m == j).astype(np.float32)  # replicated 32x32 identity
    return consts


def _shard_core(am_rows: np.ndarray) -> np.ndarray:
    """[8, 16, 65536] f32 -> [P, C_PER_P, G, GR] fp8 device layout."""
    import ml_dtypes

    ndt = ml_dtypes.float8_e4m3
    xr = np.empty((G, GR, N), dtype=ndt)
    xr[:, 0 : 2 * K, :] = am_rows.reshape(G, 2 * K, N).astype(ndt)
    xr[:, 2 * K, :] = 1.0
    # pixel n = p*C_PER_P + c ; [G, GR, p, c] -> [p, c, G, GR]
    xt = xr.reshape(G, GR, P, C_PER_P).transpose(2, 3, 0, 1)
    return np.ascontiguousarray(xt)


def kernel(am: np.ndarray) -> np.ndarray:
    global LAST_RESULTS
    from concourse.bass_utils import run_bass_kernel_spmd

    if "nc" not in _CACHE:
        _CACHE["nc"] = _build_nc()
        _CACHE["consts"] = _make_consts()
    nc = _CACHE["nc"]
    consts = _CACHE["consts"]

    am = np.ascontiguousarray(np.asarray(am), dtype=np.float32)
    assert am.shape == (B, K, N)

    in_maps = []
    for core in range(NCORES):
        rows = am[core * BPC : (core + 1) * BPC]
        in_maps.append({"x": _shard_core(rows), "consts": consts})

    trace = bool(int(os.environ.get("KERNEL_TRACE", "0")))
    res = run_bass_kernel_spmd(
        nc, in_maps, core_ids=list(range(NCORES)), trace=trace
    )
    LAST_RESULTS = res

    tsum = float(
        np.sum(
            np.array([r["out"][:, 0] for r in res.results], dtype=np.float64)
        )
    )
    npairs = K * (K - 1) // 2
    ntot = B * npairs  # 7680 masked pairs overall
    loss = (2.0 * tsum + SMOOTH * ntot) / (2.0 + SMOOTH) / ntot
    return np.float32(loss)


# revision 6
# speedup vs baseline: 1.9113x; 1.9113x over previous
"""Trainium2 Bass kernel for pairwise DiceLoss (v3).

Math (per reference):
    an[b,k,:]  = am[b,k,:] / (S[b,k] + EPS),  S = row sums of am
    gram_n     = an . an^T per batch          (16 x 16 per batch)
    dice[b,k,l]= (2*gram_n + 0.1) / (a[b,k] + a[b,l] + 0.1),  a = S/(S+EPS)
    loss       = mean over masked (k<l, same batch) pairs and batches

fp32-exact algebra: S ~ 32768 so S + 1e-8 == S in fp32 => a == 1.0 exactly
and the dice denominator is the constant 2.1 (identical to the reference's
own fp32 arithmetic to ~1e-7).  The device returns only the masked
normalized-gram row sums t_m = sum_j mask*G[m,j]*r_m*r_j; host applies the
affine map to the loss.

Device strategy (per core, 8 batches x 16 slots = 128 rows = 128 SBUF
partitions; measured-driven, see v1/v2 history):
  - One full 128-row Gram via 512 accumulating PE matmuls (K=128 pixels per
    chunk).  PE issue floor is ~34ns per LDWEIGHTS+MATMUL pair, so fewer,
    wider matmuls win: tile_position splits (v2) measured 2x WORSE.
  - fp8e4m3 input (4x less HBM traffic; error cancels over 65536-element
    contractions).  The rhs stream (1 col/cycle @2.4GHz) is the binding
    resource at ~53ns/128-pixel chunk.
  - Rows reordered so the 8 slot-0 rows come first: a column j is needed
    only for pairs m<j in the same batch, so slot-0 columns produce nothing
    -> rhs streams only columns 8..128 (120 data + ones), N=121 not 129.
  - Warmup: ~28 junk matmuls with no DMA dependency issue from program
    start, carrying the PE through the HAM activity window so real matmuls
    run at 2.4GHz from the first tile (measured: removes ~5us cold penalty).
  - DMA: few large tiles sized so the serial dma_start issue rate (~0.66us
    each on the Sync queue) stays ahead of the ~0.38MB/us 16-engine drain.
  - Epilogue: r = 1/S per row (EPS is below fp32 ulp of S), one bf16
    partition-broadcast matmul against a permuted identity gives
    rB[p,j]=r[row(j)], then 3 DVE ops + row-reduce; output is [128,1] f32.
Host: loss = (2*sum(t) + 0.1*npairs_total) / 2.1 / npairs_total.

Measured on 8 axon TRN2 cores: see test.py output.
"""

import os

import numpy as np

B, K, N = 64, 16, 65536
NCORES = 8
BPC = B // NCORES  # 8 batches per core
R = BPC * K  # 128 data rows per core
P = 128  # SBUF partitions
C_PER_P = N // P  # 512 pixel-chunks of 128
NC = R - BPC + 1  # 121 streamed columns: 120 slot>0 rows + ones
TILES = [24, 64, 104, 104, 108, 108]  # sums to C_PER_P
WARMUP = int(os.environ.get("KERNEL_WARMUP", "28"))
SMOOTH = 0.1

_CACHE: dict = {}

# test.py reads this after calling kernel() to print HW exec time
LAST_RESULTS = None


def _row_order() -> np.ndarray:
    """Row permutation: the 8 slot-0 rows first, then slot 1..15 by batch."""
    first = [b * K for b in range(BPC)]
    rest = [b * K + k for b in range(BPC) for k in range(1, K)]
    return np.array(first + rest)


def _build_nc():
    import concourse.bacc as bacc
    import concourse.mybir as mybir
    import concourse.tile as tile

    f32 = mybir.dt.float32
    bf16 = mybir.dt.bfloat16
    xdt = mybir.dt.float8e4
    nc = bacc.Bacc("TRN2", target_bir_lowering=False)

    x = nc.dram_tensor("x", [P, C_PER_P, R + 1], xdt, kind="ExternalInput")
    consts = nc.dram_tensor("consts", [P, 2 * NC], bf16, kind="ExternalInput")
    out_d = nc.dram_tensor("out", [P, 1], f32, kind="ExternalOutput")

    with tile.TileContext(nc) as tc:
        with (
            tc.tile_pool(name="xp", bufs=1) as xp,
            tc.tile_pool(name="sg", bufs=1) as sg,
            tc.tile_pool(name="ps", bufs=1, space="PSUM") as ps,
            tc.tile_pool(name="ps2", bufs=1, space="PSUM") as ps2,
            tc.tile_pool(name="wps", bufs=1, space="PSUM") as wps,
        ):
            # ---- PE warmup: junk matmuls with no DMA dependency ----
            wjunk = sg.tile([P, 16], xdt, name="wjunk")
            nc.gpsimd.memset(wjunk[:], 0.5)
            w_ps = wps.tile([P, 128], f32)
            for _ in range(WARMUP):
                nc.tensor.matmul(
                    w_ps[0:16, :],
                    wjunk[:, 0:16],
                    wjunk[:, 0:1].to_broadcast([P, 128]),
                    start=True,
                    stop=True,
                )

            # ---- input tiles (all resident; 66KB/partition at fp8) ----
            xts = []
            off = 0
            for t, cc in enumerate(TILES):
                xt = xp.tile([P, cc, R + 1], xdt, name=f"xt{t}")
                nc.sync.dma_start(xt[:], x[:, off : off + cc, :])
                xts.append((xt, off, cc))
                off += cc
            # epilogue-only data, off the critical path
            consts_sb = sg.tile([P, 2 * NC], bf16)
            nc.sync.dma_start(consts_sb[:], consts[:, :])
            maskc = consts_sb[:, 0:NC]
            identc = consts_sb[:, NC : 2 * NC]

            # ---- Gram accumulation: 512 x (lhsT [128,128], rhs [128,121]) --
            g_ps = ps.tile([P, NC], f32)
            for xt, off, cc in xts:
                for c in range(cc):
                    nc.tensor.matmul(
                        g_ps[:],
                        xt[:, c, 0:R],
                        xt[:, c, BPC : R + 1],
                        start=(off + c == 0),
                        stop=(off + c == C_PER_P - 1),
                    )

            # ---- epilogue ----
            s_ps = g_ps[:, NC - 1 : NC]  # S[row] in PSUM (ones column)
            pack = sg.tile([P, 1], f32)
            nc.vector.reciprocal(pack[:], s_ps)  # r = 1/S
            rcol = pack[:, 0:1]
            rcolb = sg.tile([P, 1], bf16)
            nc.vector.tensor_copy(out=rcolb[:], in_=rcol)

            # rB[p, j] = r[row(j)]: partition broadcast via one bf16 matmul
            # lhsT[p, m] = r[p] (step-0 free AP), rhs = permuted identity
            rB = ps2.tile([P, NC], f32)
            nc.tensor.matmul(
                rB[:],
                rcolb[:, 0:1].to_broadcast([P, P]),
                identc,
                start=True,
                stop=True,
            )

            t1 = sg.tile([P, NC], f32)
            nc.vector.tensor_scalar_mul(t1[:], g_ps[:], rcol)  # G*r_m
            nc.vector.tensor_mul(t1[:], t1[:], rB[:])  # *r_j
            nc.vector.tensor_mul(t1[:], t1[:], maskc)  # mask k<l same batch
            osb = sg.tile([P, 1], f32)
            nc.vector.reduce_sum(osb[:], t1[:], axis=mybir.AxisListType.X)
            nc.sync.dma_start(out_d[:, :], osb[:])

    nc.compile()
    return nc


def _make_consts() -> np.ndarray:
    """[P, 2*NC] bf16: pair mask | permuted identity (0/1 exact in bf16)."""
    import ml_dtypes

    order = _row_order()  # row index of weight column m
    consts = np.zeros((P, 2 * NC), dtype=ml_dtypes.bfloat16)
    m_row = order  # [128] original row id per out partition
    j_row = order[BPC:]  # [120] original row id per streamed data column
    mb, mk = m_row // K, m_row % K
    jb, jk = j_row // K, j_row % K
    mask = (mb[:, None] == jb[None, :]) & (mk[:, None] < jk[None, :])
    consts[:, 0 : NC - 1] = mask.astype(ml_dtypes.bfloat16)
    # identity: ident[p, j] = 1 iff weight column p is streamed column j
    for j in range(NC - 1):
        consts[BPC + j, NC + j] = 1.0
    return consts


def _shard_core(am_rows: np.ndarray) -> np.ndarray:
    """[8, 16, 65536] f32 -> [P, C_PER_P, R+1] fp8 device layout."""
    import ml_dtypes

    ndt = ml_dtypes.float8_e4m3
    xr = np.empty((R + 1, N), dtype=ndt)
    xr[0:R] = am_rows.reshape(R, N)[_row_order()].astype(ndt)
    xr[R] = 1.0
    # pixel n = p*C_PER_P + c ; [bk, p, c] -> [p, c, bk]
    xt = xr.reshape(R + 1, P, C_PER_P).transpose(1, 2, 0)
    return np.ascontiguousarray(xt)


def kernel(am: np.ndarray) -> np.ndarray:
    global LAST_RESULTS
    from concourse.bass_utils import run_bass_kernel_spmd

    if "nc" not in _CACHE:
        _CACHE["nc"] = _build_nc()
        _CACHE["consts"] = _make_consts()
    nc = _CACHE["nc"]
    consts = _CACHE["consts"]

    am = np.ascontiguousarray(np.asarray(am), dtype=np.float32)
    assert am.shape == (B, K, N)

    in_maps = []
    for core in range(NCORES):
        rows = am[core * BPC : (core + 1) * BPC]
        in_maps.append({"x": _shard_core(rows), "consts": consts})

    trace = bool(int(os.environ.get("KERNEL_TRACE", "0")))
    res = run_bass_kernel_spmd(
        nc, in_maps, core_ids=list(range(NCORES)), trace=trace
    )
    LAST_RESULTS = res

    tsum = float(
        np.sum(
            np.array([r["out"][:, 0] for r in res.results], dtype=np.float64)
        )
    )
    npairs = K * (K - 1) // 2
    ntot = B * npairs  # 7680 masked pairs overall
    loss = (2.0 * tsum + SMOOTH * ntot) / (2.0 + SMOOTH) / ntot
    return np.float32(loss)
